# revision 52
# baseline (speedup 1.0000x reference)
"""Trainium2 Bass kernel for nn_FISLayerParameterSharingV2.

Math: dcumsum along an axis with discount d is multiplication by a lower
triangular matrix L[i,j] = d^(i-j).  With H = W = 128 the per-(b,t) chain is

    s3  = Ls Z3 Ls^T          (Ls strict lower triangular)
    s2  = Ls (Z2*s3) Ls^T
    out = L  (Z1*s2) L^T      (L inclusive lower triangular)

All work runs in transposed space [w partitions, (t,h) free]; B is sharded
4 per core over 8 cores (x converted to f16 on the host so the loads go
through the plain HWDGE path):

  * einsum-F: matmul(lhsT=x[b,:,h,:] (c,w), rhs=alphaT (c,3T)) emits Z^T
    tiles [w, 3T] per (b,h); 5 h-slices per one-bank PSUM tile, evacuated
    by Activation into per-b SBUF buffers Bp[w, (k,t,h)] (fp16).
  * per stage, scan-FIRST: the h-direction discounted inclusive scan
    (tensor_tensor_scan, 0-reset at t boundaries) runs on the SBUF operand
    so it is legal on GpSimd (which cannot touch PSUM); the w-direction
    left-multiply is a PE matmul with the CONSTANT stationary Ls^T
    (discount uniform across t => one matrix for all t); the elementwise
    mul consumes the matmul PSUM directly on DVE.  Strictness = a one-
    column-shifted read of the scan output into the matmul with junk
    t-boundary columns memset, d factors folded into alpha_1.
  * final stage is mm-first (q = L_W m1^T, constant L^T stationary) then a
    DVE scan that writes the f16 output tile directly -- no final PSUM
    evacuation.  Output leaves transposed as [b, w, t, h] f16; the host
    transposes and applies the 2^20 unscale.
  * engine balance: Act = einsum evacuation, GpSimd = the two SBUF scans
    per unit, DVE = the three PSUM consumers (2 muls + final scan),
    PE = einsum + 3 stage matmuls; b-major phases overlap stages(b) with
    einsum+evac(b+1).

All discount-dependent values (stationaries, scan mask) are runtime input
tensors, so the compiled program is independent of the input values.
"""

import sys
import numpy as np

for _p in ("/opt/trn_rl_repo",):
    if _p not in sys.path:
        sys.path.insert(0, _p)

B, T, C, H, W = 32, 32, 64, 128, 128
NCORES = 8
BPC = B // NCORES          # batches per core
KA = 3                     # number of alphas
C1, C2, C3 = 2.0 ** -8, 2.0 ** -6, 2.0 ** -6
UNSCALE = 1.0 / (C1 * C2 * C3)

_CACHE = {}


def _build_module():
    import concourse.bass as bass
    import concourse.mybir as mybir
    import concourse.tile as tile
    from concourse import bacc
    from contextlib import ExitStack

    dt = mybir.dt
    f32, f16 = dt.float32, dt.float16

    nc = bacc.Bacc(
        "TRN2", target_bir_lowering=False, debug=False, num_devices=NCORES
    )
    xs = nc.declare_dram_parameter("xs", [BPC, C, H, W], f16, isOutput=False)
    alphaT = nc.declare_dram_parameter("alphaT", [C, KA * T], f16, isOutput=False)
    statT = nc.declare_dram_parameter("statT", [128, 128], f16, isOutput=False)
    frhs = nc.declare_dram_parameter("frhs", [128, 128], f16, isOutput=False)
    dmask = nc.declare_dram_parameter("dmask", [128, T * 16], f32, isOutput=False)
    outT = nc.declare_dram_parameter("outT", [BPC, W, T, H], f16, isOutput=True)

    HB = 32                    # h-block size for x streaming DMA
    NHB = H // HB              # 4 h-blocks
    NG = T // 4                # 8 t-quad groups per b
    NEU = 26                   # einsum evac units per b (25x5h + 1x3h)
    MULT = mybir.AluOpType.mult
    ADD = mybir.AluOpType.add

    with tile.TileContext(nc) as tc, ExitStack() as ctx:
        const_pool = ctx.enter_context(tc.tile_pool(name="const", bufs=1))
        xpool = ctx.enter_context(tc.tile_pool(name="xp", bufs=4))
        bppool = ctx.enter_context(tc.tile_pool(name="bp", bufs=3))
        ypool = ctx.enter_context(tc.tile_pool(name="yp", bufs=6))
        mpool = ctx.enter_context(tc.tile_pool(name="mp", bufs=6))
        opool = ctx.enter_context(tc.tile_pool(name="op", bufs=6))
        epool = ctx.enter_context(
            tc.tile_pool(name="eps", bufs=4, space="PSUM")
        )
        spool = ctx.enter_context(
            tc.tile_pool(name="sps", bufs=4, space="PSUM")
        )

        alpha_t = const_pool.tile([C, KA * T], f16, tag="alpha", name="alpha")
        nc.scalar.dma_start(alpha_t[:], alphaT[:])
        stat_t = const_pool.tile([128, 128], f16, tag="stat", name="stat")
        nc.scalar.dma_start(stat_t[:], statT[:])
        fr_t = const_pool.tile([128, 128], f16, tag="fr", name="fr")
        nc.scalar.dma_start(fr_t[:], frhs[:])
        dm_t = const_pool.tile([128, T * 16], f32, tag="dm", name="dm")
        nc.scalar.dma_start(dm_t[:], dmask[:])

        # ---- x streaming: per (b, hb) one DMA [64c, HB*W] f16, b-major so
        # b0's stages start as early as possible ----
        xt_tiles = {}

        def load_x(b, hb, eng=None, split=1):
            xt = xpool.tile([C, HB * W], f16, tag="x", name=f"x{b}_{hb}")
            hs = HB // split
            for s in range(split):
                src = xs[b, :, hb * HB + s * hs : hb * HB + (s + 1) * hs, :]
                (eng or nc.sync).dma_start(
                    xt[:, s * hs * W : (s + 1) * hs * W],
                    src.rearrange("c h w -> c (h w)"),
                )
            xt_tiles[(b, hb)] = xt

        # ---- per-b z buffers Bp[w, (k,t,h)] f16 ----
        bp_tiles = {}

        def make_bp(b):
            t_ = bppool.tile([128, KA * T * 128], f16, tag="bp", name=f"bp{b}")
            bp_tiles[b] = t_

        def bpv(b):
            return bp_tiles[b][:].rearrange("p (k t h) -> p k t h", k=KA, t=T)

        # ---- einsum units: per (b, j) 5 h-slices -> one-bank [128, 512]
        # PSUM tile; bufs=4 (+ the idle stage pool during b0's head) gives
        # PE run-ahead over the Act-paced evacuations. ----
        NEU = 26                   # einsum units per b (25x5h + 1x3h)

        def einsum_unit(b, j, evac_engine, pool_rr=False):
            h0, nh = 5 * j, (5 if j < 25 else 3)
            pl = spool if (pool_rr and j % 2 == 1) else epool
            pt = pl.tile([128, 512], f32, tag="ep" if pl is epool else "sp",
                         name=f"ep{b}_{j}")
            for i in range(nh):
                h = h0 + i
                hb, hr = h // HB, h % HB
                xt = xt_tiles[(b, hb)]
                nc.tensor.matmul(
                    pt[:, i * 96 : (i + 1) * 96],
                    lhsT=xt[:, hr * W : (hr + 1) * W],
                    rhs=alpha_t[:],
                    skip_group_check=True,
                )
            src_ap = pt[:, 0 : nh * 96].rearrange(
                "p (j k t) -> p j k t", j=nh, k=KA
            )
            dst_ap = (
                bpv(b)[:, :, :, h0 : h0 + nh].rearrange("p k t j -> p j k t")
            )
            # GPSIMD cannot touch PSUM: evac engines are Act/DVE only.
            if evac_engine == "act":
                nc.scalar.copy(dst_ap, src_ap)
            else:
                nc.vector.tensor_copy(dst_ap, src_ap)

        # ---- stage ops per (b, g) ----
        # mm-first: the w-direction left-multiply (constant Ls^T stationary)
        # runs on PE, the h-direction discounted scan consumes the PSUM on
        # DVE (scans are DVE-only opcodes; GpSimd also cannot touch PSUM),
        # and the elementwise muls run SBUF-to-SBUF, mostly on GpSimd.
        # Strictness = one-column-shifted read of the scan output at the
        # mul, with Bp's z1/z2 h=0 columns pre-zeroed and the d factors
        # folded into alpha_1.
        live = {}

        def muleng(b, g, which):
            # ~1 in 6 muls on DVE to balance GpSimd's 0.42-efficiency rate
            return nc.vector if (2 * g + which + b) % 6 == 0 else nc.gpsimd

        def s3mm(b, g):
            v = bpv(b)
            p3 = spool.tile([128, 512], f32, tag="sp", name=f"p3_{b}_{g}")
            nc.tensor.matmul(
                p3[:], lhsT=stat_t[:],
                rhs=v[:, 2, 4 * g : 4 * g + 4, :].rearrange("p t h -> p (t h)"),
                skip_group_check=True,
            )
            live[(b, g, "p3")] = p3

        def scan3(b, g):
            p3 = live.pop((b, g, "p3"))
            y3 = ypool.tile([128, 516], f16, tag="y", name=f"y3_{b}_{g}")
            nc.vector.tensor_tensor_scan(
                y3[:, 1:513], dm_t[:], p3[:],
                initial=0.0, op0=MULT, op1=ADD,
            )
            live[(b, g, "y3")] = y3

        def mul2(b, g):
            v = bpv(b)
            y3 = live.pop((b, g, "y3"))
            m2 = mpool.tile([128, 512], f16, tag="m", name=f"m2_{b}_{g}")
            muleng(b, g, 0).tensor_mul(
                m2[:].rearrange("p (t h) -> p t h", t=4),
                v[:, 1, 4 * g : 4 * g + 4, :],
                y3[:, 0:512].rearrange("p (t h) -> p t h", t=4),
            )
            live[(b, g, "m2")] = m2

        def s2mm(b, g):
            m2 = live.pop((b, g, "m2"))
            p2 = spool.tile([128, 512], f32, tag="sp", name=f"p2_{b}_{g}")
            nc.tensor.matmul(
                p2[:], lhsT=stat_t[:], rhs=m2[:], skip_group_check=True
            )
            live[(b, g, "p2")] = p2

        def scan2(b, g):
            p2 = live.pop((b, g, "p2"))
            y2 = ypool.tile([128, 516], f16, tag="y", name=f"y2_{b}_{g}")
            nc.vector.tensor_tensor_scan(
                y2[:, 1:513], dm_t[:], p2[:],
                initial=0.0, op0=MULT, op1=ADD,
            )
            live[(b, g, "y2")] = y2

        def mul1(b, g):
            v = bpv(b)
            y2 = live.pop((b, g, "y2"))
            m1 = mpool.tile([128, 512], f16, tag="m", name=f"m1_{b}_{g}")
            muleng(b, g, 1).tensor_mul(
                m1[:].rearrange("p (t h) -> p t h", t=4),
                v[:, 0, 4 * g : 4 * g + 4, :],
                y2[:, 0:512].rearrange("p (t h) -> p t h", t=4),
            )
            live[(b, g, "m1")] = m1

        def fmm(b, g):
            m1 = live.pop((b, g, "m1"))
            q = spool.tile([128, 512], f32, tag="sp", name=f"q_{b}_{g}")
            nc.tensor.matmul(
                q[:], lhsT=fr_t[:], rhs=m1[:], skip_group_check=True
            )
            live[(b, g, "q")] = q

        def scanF_dma(b, g):
            q = live.pop((b, g, "q"))
            ot = opool.tile([128, 512], f16, tag="o", name=f"o_{b}_{g}")
            nc.vector.tensor_tensor_scan(
                ot[:], dm_t[:], q[:], initial=0.0, op0=MULT, op1=ADD,
            )
            dst = outT[b, :, 4 * g : 4 * g + 4, :]
            nc.sync.dma_start(dst.rearrange("w t h -> w (t h)"), ot[:])

        def zero_shift_cols(b):
            # zero Bp[:, k, t, 0] for k in {0,1} (z1, z2): the strict-shift
            # boundary, so the shifted y-reads need no per-unit memsets.
            nc.gpsimd.memset(bpv(b)[:, 0:2, :, 0:1], 0.0)

        # ---- stage tick pipeline for one b, with optional interleave.
        # Offsets: s3mm+0 scan3+1 mul2+2 s2mm+3 scan2+4 mul1+5 fmm+6 scanF+7;
        # every op's dependency resolves in an earlier tick. ----
        def stage_ticks(b, extra=None):
            for k in range(NG + 8):
                if 0 <= k - 7 < NG:
                    scanF_dma(b, k - 7)
                if 0 <= k - 1 < NG:
                    scan3(b, k - 1)
                if 0 <= k - 4 < NG:
                    scan2(b, k - 4)
                if 0 <= k - 2 < NG:
                    mul2(b, k - 2)
                if 0 <= k - 5 < NG:
                    mul1(b, k - 5)
                if k < NG:
                    s3mm(b, k)
                if 0 <= k - 3 < NG:
                    s2mm(b, k - 3)
                if 0 <= k - 6 < NG:
                    fmm(b, k - 6)
                # einsum extras LAST: their PSUM-alloc waits must not
                # head-of-line-block this tick's stage matmuls.
                if extra is not None:
                    for _ in range(3 if k < 6 else 2):
                        u_ = next(extra, None)
                        if u_ is not None:
                            u_()

        # ================= schedule =================
        load_x(0, 0, split=4)
        for hb in range(1, NHB):
            load_x(0, hb, split=2)
        for hb in range(NHB):
            load_x(1, hb)

        # pre-zero the scan-shift column of every scan buffer once; the
        # scans only ever write cols 1:513, so the zeros persist across
        # buffer reuse.
        yz = [
            ypool.tile([128, 516], f16, tag="y", name=f"yz{i}")
            for i in range(6)
        ]
        for t_ in yz:
            nc.vector.memset(t_[:, 0:1], 0.0)

        # b0: einsum + evac split Act/DVE (GPSIMD cannot read PSUM),
        # borrowing the idle stage pool for PSUM ring depth.
        make_bp(0)
        einsum_unit(0, 0, "act", pool_rr=True)
        zero_shift_cols(0)
        for j in range(1, NEU):
            einsum_unit(0, j, ("act", "dve")[j % 2], pool_rr=True)

        def ein_units(b):
            # all on Act: DVE carries the three scans per unit.
            us = [lambda: einsum_unit(b, 0, "act"),
                  lambda: zero_shift_cols(b)]
            us += [lambda j=j: einsum_unit(b, j, "act") for j in range(1, NEU)]
            return iter(us)

        make_bp(1)
        for hb in range(NHB):
            load_x(2, hb)
        ein1 = ein_units(1)
        stage_ticks(0, extra=ein1)
        for u_ in ein1:
            u_()

        make_bp(2)
        for hb in range(NHB):
            load_x(3, hb)
        ein2 = ein_units(2)
        stage_ticks(1, extra=ein2)
        for u_ in ein2:
            u_()

        make_bp(3)
        ein3 = ein_units(3)
        stage_ticks(2, extra=ein3)
        for u_ in ein3:
            u_()

        stage_ticks(3)

    nc.compile()
    return nc


def _host_prep(alpha_1, alpha_2, alpha_3, discount):
    ds = np.asarray(discount, dtype=np.float64).reshape(T)
    d = float(ds[0])   # uniform discount across t (reference uses d*ones)
    # the two strict-shift d factors fold into alpha_1 (linear downstream)
    a1 = alpha_1.T * (C1 * d * d)
    alphaT = np.concatenate([a1, alpha_2.T * C2, alpha_3.T * C3], axis=1)
    alphaT_dup = np.concatenate([alphaT, alphaT], axis=0).astype(np.float16)

    idx = np.arange(H)
    E = idx[:, None] - idx[None, :]
    P = d ** np.maximum(E, 0)
    Ls = np.where(E >= 1, P, 0.0)       # strict
    L = np.where(E >= 0, P, 0.0)        # inclusive
    statT = Ls.T.astype(np.float16).copy()
    frhs = L.T.astype(np.float16).copy()
    dmask = np.full((128, T * 16), np.float32(d), dtype=np.float32)
    dmask[:, 0::128] = 0.0              # reset at each t block start
    return alphaT_dup, statT, frhs, dmask


def kernel(x, alpha_1, alpha_2, alpha_3, discount):
    from concourse.bass_utils import run_bass_kernel_spmd

    x = np.ascontiguousarray(np.asarray(x, dtype=np.float32)).astype(np.float16)
    alphaT_dup, statT, frhs, dmask = _host_prep(
        np.asarray(alpha_1, np.float32),
        np.asarray(alpha_2, np.float32),
        np.asarray(alpha_3, np.float32),
        discount,
    )

    key = ("nc", 1)
    if key not in _CACHE:
        _CACHE[key] = _build_module()
    nc = _CACHE[key]

    shared = {
        "alphaT": alphaT_dup,
        "statT": statT,
        "frhs": frhs,
        "dmask": dmask,
    }
    in_maps = [
        {"xs": x[i * BPC : (i + 1) * BPC], **shared} for i in range(NCORES)
    ]
    res = run_bass_kernel_spmd(nc, in_maps, core_ids=list(range(NCORES)))
    outs = [res.results[i]["outT"] for i in range(NCORES)]
    full = np.concatenate(outs, axis=0)            # [B, W, T, H] f16
    out = full.transpose(0, 2, 3, 1).astype(np.float32) * np.float32(UNSCALE)
    return np.ascontiguousarray(out)


if __name__ == "__main__":
    import reference as ref

    inputs = {k: np.asarray(v) for k, v in ref.setup_inputs().items()}
    got = kernel(**inputs)
    print("kernel output shape:", got.shape, got.dtype)
